# revision 4
# baseline (speedup 1.0000x reference)
"""Trainium2 Bass kernel for a Conv-TasNet-style decoder (mask * wave ->
overlap_and_add -> trim).

Reference computation (per batch element b):
    A[c, d, t] = x[b, c, d, t] * x_wave[b, d, t]          (broadcast over c)
    frames     = A transposed to [c, t, d]  (frame length D=16, hop 8)
    unsliced   = overlap_and_add(frames, 8)               # [c, (T+1)*8]
    y          = unsliced[:, pad_left : -pad_right]

With hop=8 and D=16, overlap_and_add decomposes into two interleaved
streams, and for the middle region (everything when pads are 8):

    y[c][8s + r] = x[c, r, s+1]*w[r, s+1] + x[c, r+8, s]*w[r+8, s]

i.e. purely elementwise over s plus an 8-way interleave.  The device
kernel computes this on a [128 partitions x 8000] grid (partition p
owns frames [p*1000, (p+1)*1000)); the +1 frame shift is baked into
the DMA-load access patterns (flat-offset views) and the (s, r)
interleave into the final add's write access pattern.  The last 8
elements of the [2, 1024000] padded device output are garbage (frame
index T) and are trimmed on the host.

Schedule notes (all measured on HW traces):
 - The kernel is DMA-engine-time bound; per-engine throughput rises
   with descriptor (contiguous run) size: 512B -> 16.5 B/ns, 1000B ->
   20-22, 2000B -> 24.4, 8KB -> 26.5.  Load runs are fc*4 bytes, so
   the middle chunk is loaded as one fc=500 DMA (2000B runs) and
   computed in two 250-frame sub-slices; the first chunk is small
   (fc=128) purely to shorten the pipeline ramp, and the last (fc=122)
   to shorten the serial tail.
 - W is loaded per-chunk, interleaved with the x stream in exact
   compute order (a bulk W load starves the vector engine).
 - Chunk order k-outer / speaker-inner so each W chunk is used twice
   then freed; the last two iterations are swapped (k3c0 before k2c1)
   so only one small iteration remains after the final load.
 - Low-side loads ride the SP HWDGE queue, high-side the ACT queue,
   stores the Pool SWDGE queue (8KB runs).

Sharding: pure data parallel -- core b computes batch element b (B=8
matches the 8 NeuronCores); no cross-core communication.
"""

import numpy as np

_B, _C, _D, _T = 8, 2, 16, 128000
_HOP = 8
_S = _T * _HOP            # padded per-speaker device output length (1024000)
_MID = _S - _HOP          # valid middle length (1023992)
_P = 128                  # SBUF partitions
_JB = _T // _P            # frames per partition block (1000)

# Load chunks tiling the 1000-frame block, with compute sub-slices.
_CHUNKS = [(0, 128), (128, 250), (378, 500), (878, 122)]
_SUBS = {500: [(0, 250), (250, 250)]}   # split large chunks for compute
_FCMAX = 512

_cached = None            # (nc, run_bass_kernel_spmd)


def _build():
    """Build the Bass module (one NeuronCore's program). Cached."""
    global _cached
    if _cached is not None:
        return _cached

    import concourse.bacc as bacc
    import concourse.mybir as mybir
    import concourse.tile as tile
    from concourse.bass_utils import run_bass_kernel_spmd

    f32 = mybir.dt.float32
    T, P = _T, _P

    nc = bacc.Bacc(debug=False)
    x = nc.declare_dram_parameter("x", [_C, _D, T], f32, isOutput=False)
    w = nc.declare_dram_parameter("x_wave", [_D, T], f32, isOutput=False)
    y = nc.declare_dram_parameter("y_pad", [_C, _S], f32, isOutput=True)

    # Flat 1-D views let us bake the +1-frame shift into the AP offset
    # (a shifted [r, s] view crosses row boundaries, which plain
    # slice-then-rearrange cannot express).
    xf = x[:].rearrange("c d t -> (c d t)")
    wf = w[:].rearrange("d t -> (d t)")
    yf = y[:].rearrange("c n -> (c n)")

    def rpj(flat, start):
        # [p, r, j] view: element = flat[start + r*T + p*_JB + j]
        return flat[start : start + 8 * T].rearrange("(r p j) -> p r j", r=8, p=P)

    wl_full = rpj(wf, 1)          # w[r, s+1]
    wh_full = rpj(wf, 8 * T)      # w[r+8, s]
    xv = []                       # per speaker: (xl view, xh view, y view)
    for c in range(_C):
        base = c * _D * T
        xv.append(
            (
                rpj(xf, base + 1),       # x[c, r, s+1]
                rpj(xf, base + 8 * T),   # x[c, r+8, s]
                yf[c * _S : (c + 1) * _S].rearrange("(p q) -> p q", p=P),
            )
        )

    # (chunk_idx, speaker) iteration order: k-outer / c-inner.  Chunk
    # order is big-in-the-middle, small-last: the ramp chunk first,
    # then the large fc=500 chunk (so its bulk loads and its 4 compute
    # sub-iterations drain mid-stream where the pipeline hides them),
    # and the small chunks last so only ~4us of compute trails the
    # final load.
    order = []
    for k in (0, 2, 1, 3):
        for c in range(_C):
            order.append((k, c))

    with tile.TileContext(nc) as tc:
        with (
            tc.tile_pool(name="wpool", bufs=2) as wpool,
            tc.tile_pool(name="xpool", bufs=3) as xpool,
            tc.tile_pool(name="ppool", bufs=1) as ppool,
            tc.tile_pool(name="zpool", bufs=3) as zpool,
        ):
            w_tiles = {}   # k -> (wlt, wht)
            x_tiles = {}   # (k, c) -> (xlt, xht)
            loaded_w = set()
            loaded_x = set()

            def load_w(k):
                if k in loaded_w or k >= len(_CHUNKS):
                    return
                loaded_w.add(k)
                j0, fc = _CHUNKS[k]
                wlt = wpool.tile([P, 8, _FCMAX], f32, tag="wl", name="wlt")[:, :, :fc]
                nc.sync.dma_start(out=wlt[:], in_=wl_full[:, :, j0 : j0 + fc])
                wht = wpool.tile([P, 8, _FCMAX], f32, tag="wh", name="wht")[:, :, :fc]
                nc.scalar.dma_start(out=wht[:], in_=wh_full[:, :, j0 : j0 + fc])
                w_tiles[k] = (wlt, wht)

            def load_x(k, c):
                if (k, c) in loaded_x:
                    return
                loaded_x.add((k, c))
                j0, fc = _CHUNKS[k]
                xl_full, xh_full, _ = xv[c]
                xlt = xpool.tile([P, 8, _FCMAX], f32, tag="xl", name="xlt")[:, :, :fc]
                nc.sync.dma_start(out=xlt[:], in_=xl_full[:, :, j0 : j0 + fc])
                xht = xpool.tile([P, 8, _FCMAX], f32, tag="xh", name="xht")[:, :, :fc]
                nc.scalar.dma_start(out=xht[:], in_=xh_full[:, :, j0 : j0 + fc])
                x_tiles[(k, c)] = (xlt, xht)

            # Issue loads in compute-need order (queues are FIFO).
            for k, c in order:
                load_w(k)
                load_x(k, c)

            for k, c in order:
                j0, fc = _CHUNKS[k]
                wlt, wht = w_tiles[k]
                xlt, xht = x_tiles[(k, c)]
                y_c = xv[c][2]
                for s0, sn in _SUBS.get(fc, [(0, fc)]):
                    # Products on DVE with contiguous APs (full rate);
                    # the add reads contiguously and scatters the
                    # (r, j) -> 8j + r interleave into its write AP.
                    yt = ppool.tile([P, 2048], f32, tag="yt", name="yt")[:, : 8 * sn]
                    tt = ppool.tile([P, 2048], f32, tag="tt", name="tt")[:, : 8 * sn]
                    zt = zpool.tile([P, 2048], f32, tag="zt", name="zt")[:, : 8 * sn]
                    sl = slice(s0, s0 + sn)
                    nc.vector.tensor_mul(yt[:], xlt[:, :, sl], wlt[:, :, sl])
                    nc.vector.tensor_mul(tt[:], xht[:, :, sl], wht[:, :, sl])
                    nc.vector.tensor_add(
                        zt.rearrange("p (j r) -> p r j", r=8),
                        yt.rearrange("p (r j) -> p r j", r=8),
                        tt.rearrange("p (r j) -> p r j", r=8),
                    )
                    o0 = 8 * (j0 + s0)
                    nc.gpsimd.dma_start(out=y_c[:, o0 : o0 + 8 * sn], in_=zt[:])

    nc.compile()  # legalize sync waits (>=1 wait/inst split into events)

    _cached = (nc, run_bass_kernel_spmd)
    return _cached


def _run_device(x, w, trace=False):
    nc, run_bass_kernel_spmd = _build()
    in_maps = [
        {"x": np.ascontiguousarray(x[b]), "x_wave": np.ascontiguousarray(w[b])}
        for b in range(_B)
    ]
    res = run_bass_kernel_spmd(nc, in_maps, core_ids=list(range(_B)), trace=trace)
    mid = np.stack([r["y_pad"][:, :_MID] for r in res.results])
    return mid, res


def kernel(x, x_wave, pad_left=8, pad_right=8, _trace=False, _return_res=False):
    x = np.asarray(x, dtype=np.float32)
    w = np.asarray(x_wave, dtype=np.float32)
    pl, pr = int(pad_left), int(pad_right)
    assert x.shape == (_B, _C, _D, _T) and w.shape == (_B, _D, _T)

    mid, res = _run_device(x, w, trace=_trace)

    if pl == 8 and pr == 8:
        out = mid
    else:
        # General trim: reconstruct the 8 leading / 8 trailing elements
        # of the unsliced overlap-add on the host (they only involve the
        # first/last frame) and slice.
        front = x[:, :, 0:8, 0] * w[:, None, 0:8, 0]        # unsliced[0:8]
        back = x[:, :, 8:16, -1] * w[:, None, 8:16, -1]     # unsliced[-8:]
        full = np.concatenate([front, mid, back], axis=-1)  # [B, C, (T+1)*8]
        end = full.shape[-1] - pr
        out = np.ascontiguousarray(full[:, :, pl:end])

    if _return_res:
        return out, res
    return out
